# revision 12
# baseline (speedup 1.0000x reference)
"""ContextAttention Trainium2 kernel (8-core data parallel).

Computation (per batch row b, S=20, D=300, J=512):
  valid = cumprod(labels != 0)                      prefix-valid mask
  fea   = guide[ann[2b]]                            (host gather, pure data movement)
  pre[s,:] = ctx[b,s,:] @ W_sent.T + b_sent + b_emb + valid[b,s]*(fea @ W_emb.T)
  H = tanh(pre);  scores = H @ w_fc                 (b_fc dropped: softmax shift-invariant)
  attn = renorm(softmax(scores) * (labels != 0))
  out[b,:] = sum_s attn[s] * embedded[b,s,:]

Device layout: s-major, 128-batch tiles. Contraction-dim-major ("transposed")
context/fea prepared on host so the PE streams natural tiles; the valid-gated
guidance add and the attn-weighted sum both run as diagonal-matrix matmuls
accumulated in PSUM.
"""

import sys
from contextlib import ExitStack

import numpy as np

if "/opt/trn_rl_repo" not in sys.path:
    sys.path.append("/opt/trn_rl_repo")

import concourse.bass as bass
import concourse.tile as tile
from concourse import bacc, mybir
from concourse.bass_utils import run_bass_kernel_spmd

B, S, D, J, VG = 8192, 20, 300, 512, 2078
NCORES = 8
BC = B // NCORES          # 1024 batch rows per core
NBT = BC // 128           # 8 batch tiles per core
DX = D + 1                # ones row appended for fused bias
VGP = 2176                # VG padded to 17*128
NK = VGP // 128           # 17 contraction chunks for fea @ W_emb.T
DP = [(0, 128), (128, 128), (256, DX - 256)]  # contraction chunks for ctx @ W_sent.T
SH = 10                   # s-halves for SBUF footprint
F32 = mybir.dt.float32

_NC_CACHE = {}


def _build(mm_dt):
    nc = bacc.Bacc("TRN2", target_bir_lowering=False, debug=False)
    MMD = mm_dt

    ctx_d = nc.dram_tensor("ctx", [NBT, DX, S, 128], MMD, kind="ExternalInput").ap()
    emb_d = nc.dram_tensor("emb", [BC, S * D], MMD, kind="ExternalInput").ap()
    fea_d = nc.dram_tensor("feaT", [NBT, 128, NK, 128], MMD, kind="ExternalInput").ap()
    lab_d = nc.dram_tensor("lab", [NBT, 128, S], F32, kind="ExternalInput").ap()
    wst_d = nc.dram_tensor("wst", [DX, J], MMD, kind="ExternalInput").ap()
    wet_d = nc.dram_tensor("wet", [NK, 128, J], MMD, kind="ExternalInput").ap()
    wfc_d = nc.dram_tensor("wfc", [128, J], F32, kind="ExternalInput").ap()
    eye_d = nc.dram_tensor("eye", [128, S * 128], MMD, kind="ExternalInput").ap()
    out_d = nc.dram_tensor("wc", [NBT, 128, D], F32, kind="ExternalOutput").ap()

    mul = mybir.AluOpType.mult
    add = mybir.AluOpType.add

    with tile.TileContext(nc) as tc, ExitStack() as ctx:
        consts = ctx.enter_context(tc.tile_pool(name="consts", bufs=1))
        ctxp = ctx.enter_context(tc.tile_pool(name="ctxp", bufs=2))
        feap = ctx.enter_context(tc.tile_pool(name="feap", bufs=2))
        fep = ctx.enter_context(tc.tile_pool(name="fep", bufs=2))
        embp = ctx.enter_context(tc.tile_pool(name="embp", bufs=1))
        hp = ctx.enter_context(tc.tile_pool(name="hp", bufs=3))
        dgp = ctx.enter_context(tc.tile_pool(name="dgp", bufs=1))
        sm = ctx.enter_context(tc.tile_pool(name="sm", bufs=2))
        outp = ctx.enter_context(tc.tile_pool(name="outp", bufs=2))
        ps_fe_p = ctx.enter_context(tc.tile_pool(name="psfe", bufs=2, space="PSUM"))
        ps_h_p = ctx.enter_context(tc.tile_pool(name="psh", bufs=2, space="PSUM"))
        ps_wc_p = ctx.enter_context(tc.tile_pool(name="pswc", bufs=2, space="PSUM"))

        wet_sb = []
        for k in range(NK):
            t = consts.tile([128, J], MMD, tag=f"wet{k}")
            nc.sync.dma_start(out=t, in_=wet_d[k])
            wet_sb.append(t)
        wst_sb = []
        for i, (o, p) in enumerate(DP):
            t = consts.tile([p, J], MMD, tag=f"wst{i}")
            nc.sync.dma_start(out=t, in_=wst_d[o : o + p])
            wst_sb.append(t)
        wfc_sb = consts.tile([128, J], F32, tag="wfc")
        nc.sync.dma_start(out=wfc_sb, in_=wfc_d)
        eye_sb = consts.tile([128, S * 128], MMD, tag="eye")
        nc.sync.dma_start(out=eye_sb, in_=eye_d)
        eye3 = eye_sb[:].rearrange("p (s q) -> p s q", s=S)

        import os

        nbt_run = int(os.environ.get("K_NBT", NBT))
        for bt in range(nbt_run):
            fea_sb = feap.tile([128, VGP], MMD, tag="feaT")
            nc.sync.dma_start(out=fea_sb, in_=fea_d[bt].rearrange("p a b -> p (a b)"))
            lab_sb = sm.tile([128, S], F32, tag="lab")
            nc.sync.dma_start(out=lab_sb, in_=lab_d[bt])
            emb_sb = embp.tile([128, S * D], MMD, tag="emb")
            nc.sync.dma_start(out=emb_sb, in_=emb_d[bt * 128 : (bt + 1) * 128])

            # fea_emb = fea @ W_emb.T  (no bias: b_emb fused into wst ones-row)
            ps_fe = ps_fe_p.tile([128, J], F32, tag="psfe")
            for k in range(NK):
                nc.tensor.matmul(
                    ps_fe,
                    fea_sb[:, k * 128 : (k + 1) * 128],
                    wet_sb[k][:],
                    start=(k == 0),
                    stop=(k == NK - 1),
                )
            fe_sb = fep.tile([128, J], MMD, tag="fe")
            nc.vector.tensor_scalar(fe_sb, ps_fe, 1.0, None, mul)
            stage = os.environ.get("K_STAGE", "full")
            if stage == "fe":
                dbg = outp.tile([128, D], F32, tag="ot")
                nc.vector.tensor_copy(dbg, fe_sb[:, 0:D])
                nc.sync.dma_start(out=out_d[bt], in_=dbg)
                continue

            # masks: nz = labels != 0 ; valid = prefix-AND(nz)
            nz = sm.tile([128, S], F32, tag="nz")
            nc.vector.tensor_scalar(nz, lab_sb, 0.0, None, mybir.AluOpType.not_equal)
            va = sm.tile([128, S], F32, tag="va")
            vb = sm.tile([128, S], F32, tag="vb")
            nc.vector.tensor_copy(va, nz)
            cur, nxt = va, vb
            for k in (1, 2, 4, 8, 16):
                nc.vector.tensor_copy(nxt[:, :k], cur[:, :k])
                nc.vector.tensor_tensor(
                    out=nxt[:, k:S], in0=cur[:, k:S], in1=cur[:, 0 : S - k], op=mul
                )
                cur, nxt = nxt, cur
            valid = cur

            # valid-diag: vd[p, s*128+q] = (p==q) * valid[p, s]
            vd = dgp.tile([128, S * 128], MMD, tag="vd")
            nc.vector.tensor_tensor(
                out=vd[:].rearrange("p (s q) -> p s q", s=S),
                in0=eye3,
                in1=valid[:].unsqueeze(2).broadcast_to([128, S, 128]),
                op=mul,
            )

            if stage == "mask":
                dbg = outp.tile([128, D], F32, tag="ot")
                nc.vector.tensor_copy(dbg, vd[:, 0:D])
                nc.sync.dma_start(out=out_d[bt], in_=dbg)
                continue
            scores = sm.tile([128, S], F32, tag="scores")
            for h in range(S // SH):
                cxs = []
                for i, (o, p) in enumerate(DP):
                    t = ctxp.tile([p, SH * 128], MMD, tag=f"cx{i}")
                    nc.sync.dma_start(
                        out=t,
                        in_=ctx_d[bt, o : o + p, h * SH : (h + 1) * SH, :].rearrange(
                            "p s b -> p (s b)"
                        ),
                    )
                    cxs.append(t)
                for si in range(SH):
                    s = h * SH + si
                    ps_h = ps_h_p.tile([128, J], F32, tag="psh")
                    for i in range(3):
                        nc.tensor.matmul(
                            ps_h,
                            cxs[i][:, si * 128 : (si + 1) * 128],
                            wst_sb[i][:],
                            start=(i == 0),
                            stop=False,
                        )
                    nc.tensor.matmul(
                        ps_h,
                        vd[:, s * 128 : (s + 1) * 128],
                        fe_sb[:],
                        start=False,
                        stop=True,
                    )
                    ht = hp.tile([128, J], F32, tag="H")
                    nc.scalar.activation(ht, ps_h, mybir.ActivationFunctionType.Tanh)
                    if stage == "tanh":
                        continue
                    hw = hp.tile([128, J], F32, tag="HW")
                    nc.vector.tensor_tensor(out=hw, in0=ht, in1=wfc_sb[:], op=mul)
                    nc.vector.tensor_reduce(
                        scores[:, s : s + 1], hw[:], axis=mybir.AxisListType.X,
                        op=add,
                    )

            if stage in ("scores", "tanh"):
                dbg = outp.tile([128, D], F32, tag="ot")
                nc.vector.memset(dbg, 0.0)
                if stage == "scores":
                    nc.vector.tensor_copy(dbg[:, 0:S], scores[:])
                nc.sync.dma_start(out=out_d[bt], in_=dbg)
                continue
            # masked softmax over s, renormalized
            negm = sm.tile([128, 1], F32, tag="negm")
            nc.vector.tensor_reduce(
                negm, scores[:], axis=mybir.AxisListType.X,
                op=mybir.AluOpType.max, negate=True,
            )
            e = sm.tile([128, S], F32, tag="e")
            nc.scalar.activation(
                e, scores[:], mybir.ActivationFunctionType.Exp, bias=negm[:, 0:1]
            )
            emk = sm.tile([128, S], F32, tag="emk")
            den = sm.tile([128, 1], F32, tag="den")
            nc.vector.tensor_tensor(out=emk, in0=e[:], in1=nz[:], op=mul)
            nc.vector.tensor_reduce(den, emk[:], axis=mybir.AxisListType.X, op=add)
            rden = sm.tile([128, 1], F32, tag="rden")
            nc.vector.reciprocal(rden, den)
            attn = sm.tile([128, S], F32, tag="attn")
            nc.vector.tensor_scalar(attn, emk, rden[:, 0:1], None, mul)

            if stage == "softmax":
                dbg = outp.tile([128, D], F32, tag="ot")
                nc.vector.memset(dbg, 0.0)
                nc.vector.tensor_copy(dbg[:, 0:S], attn[:])
                nc.sync.dma_start(out=out_d[bt], in_=dbg)
                continue
            # attn-diag + weighted sum of embedded, accumulated in PSUM
            ad = dgp.tile([128, S * 128], MMD, tag="ad")
            nc.vector.tensor_tensor(
                out=ad[:].rearrange("p (s q) -> p s q", s=S),
                in0=eye3,
                in1=attn[:].unsqueeze(2).broadcast_to([128, S, 128]),
                op=mul,
            )
            ps_wc = ps_wc_p.tile([128, D], F32, tag="pswc")
            for s in range(S):
                nc.tensor.matmul(
                    ps_wc,
                    ad[:, s * 128 : (s + 1) * 128],
                    emb_sb[:, s * D : (s + 1) * D],
                    start=(s == 0),
                    stop=(s == S - 1),
                )
            ot = outp.tile([128, D], F32, tag="ot")
            nc.scalar.copy(ot, ps_wc)
            nc.sync.dma_start(out=out_d[bt], in_=ot)

    nc.compile()
    return nc


def _get_nc(mm_dt_name="float32r"):
    if mm_dt_name not in _NC_CACHE:
        _NC_CACHE[mm_dt_name] = _build(getattr(mybir.dt, mm_dt_name))
    return _NC_CACHE[mm_dt_name]


def prep_inputs(context, embedded, input_labels, guide_input, sent_to_image_ann,
                W_sent, b_sent, W_emb, b_emb, w_fc, b_fc):
    """Host-side shard + layout prep. Pure data movement plus weight layout."""
    context = np.asarray(context, np.float32)
    embedded = np.asarray(embedded, np.float32)
    labels = np.asarray(input_labels)
    guide = np.asarray(guide_input, np.float32)
    ann2 = np.asarray(sent_to_image_ann)[::2]
    fea = guide[ann2]  # (B, VG) row gather

    wst = np.empty((DX, J), np.float32)
    wst[:D] = np.asarray(W_sent, np.float32).T
    wst[D] = np.asarray(b_sent, np.float32) + np.asarray(b_emb, np.float32)
    wet = np.zeros((VGP, J), np.float32)
    wet[:VG] = np.asarray(W_emb, np.float32).T
    wet = wet.reshape(NK, 128, J)
    wfc = np.tile(np.asarray(w_fc, np.float32)[None, :], (128, 1))
    eye = np.ascontiguousarray(
        np.tile(np.eye(128, dtype=np.float32), (1, S)).reshape(128, S * 128)
    )

    in_maps = []
    for c in range(NCORES):
        c0 = c * BC
        ctx_c = context[c0 : c0 + BC].reshape(NBT, 128, S, D).transpose(0, 3, 2, 1)
        ctx_l = np.empty((NBT, DX, S, 128), np.float32)
        ctx_l[:, :D] = ctx_c
        ctx_l[:, D] = 1.0
        fea_c = np.zeros((BC, VGP), np.float32)
        fea_c[:, :VG] = fea[c0 : c0 + BC]
        fea_l = fea_c.reshape(NBT, 128, NK, 128).transpose(0, 3, 2, 1)
        in_maps.append({
            "ctx": np.ascontiguousarray(ctx_l),
            "emb": np.ascontiguousarray(embedded[c0 : c0 + BC].reshape(BC, S * D)),
            "feaT": np.ascontiguousarray(fea_l),
            "lab": labels[c0 : c0 + BC].reshape(NBT, 128, S).astype(np.float32),
            "wst": wst, "wet": wet, "wfc": wfc, "eye": eye,
        })
    return in_maps


def kernel(**inputs):
    in_maps = prep_inputs(**inputs)
    nc = _get_nc()
    res = run_bass_kernel_spmd(nc, in_maps, list(range(NCORES)))
    return np.concatenate(
        [res.results[i]["wc"].reshape(BC, D) for i in range(NCORES)], axis=0
    )
